# revision 48
# baseline (speedup 1.0000x reference)
"""Fused cross-attention (LoRA + IP-Adapter) Trainium2 kernel, v2.

Sharding: data-parallel over (batch, seq/2) -> 8 shards of 2048 query rows.
Each core computes its shard end-to-end; no collectives. LoRA deltas and the
attention scale are folded into the projection weights on the host.

v2 design (vs v1): weight-folding on device to shrink the per-chunk loop.
  - M_h = WqT_h @ kT_h  [640, 100] per head, computed once. Then
    simT_h = M_h^T @ xT directly (5 accum matmuls) -- no per-chunk q
    projection or qT eviction.
  - VW_h = v_h @ Wout_h [100, 640] per head (bf16), computed once. Then
    y_j = sum_h at_h[:, j]^T @ VW_h -- the normalized attention weights are
    the stationary operand; no AV matmul, no outT buffer.
  - Softmax: exp -> per-(head,range) sums accumulated into one [16,512]
    PSUM tile via per-head selector matmuls -> a single
    reciprocal_approx_fast per 512-query chunk -> per-head broadcast
    matmul -> one DVE multiply producing bf16 attention weights.
  - Bias: pad context row 95 carries at==1 (selector row 16) and
    VW[95, h=0, :] = bout, so yproj accumulates the bias for free.
  - Context is embedded into an augmented [1536, 640] contraction
    (org tokens use rows 0:768 = Wk/Wv, IPA tokens rows 768:1536 =
    Wk_ipa/Wv_ipa), so one matmul pair projects k and v for all tokens.
"""
import sys, types

for _p in ("/opt/trn_rl_repo", "/root/.axon_site", "/root/.axon_site/_ro/trn_rl_repo"):
    if _p not in sys.path:
        sys.path.append(_p)

import numpy as np


def install_ntff_shim():
    """The image's antenv lacks axon_hooks; inject it and register the ctypes
    NTFF profile hook so run_bass_kernel_spmd(trace=True) yields exec_time_ns."""
    if "antenv.axon_hooks" in sys.modules:
        return
    mod = types.ModuleType("antenv.axon_hooks")
    mod._hook = None

    def set_axon_ntff_profile_hook(hook):
        mod._hook = hook

    def get_axon_ntff_profile_hook():
        return mod._hook

    mod.set_axon_ntff_profile_hook = set_axon_ntff_profile_hook
    mod.get_axon_ntff_profile_hook = get_axon_ntff_profile_hook
    sys.modules["antenv.axon_hooks"] = mod
    try:
        from trn_agent_boot.trn_boot import _ntff_profile_via_ctypes
        mod._hook = _ntff_profile_via_ctypes("/opt/axon/libaxon_pjrt.so")
    except Exception:
        pass


install_ntff_shim()

import ml_dtypes
import concourse.bass as bass
import concourse.bacc as bacc
import concourse.tile as tile
from concourse import mybir
from concourse.bass_utils import run_bass_kernel_spmd

P = 128
B, N, QD, CD = 4, 4096, 640, 768
H, DH = 8, 80
INNER = 640
ORG, IPA = 77, 4
CTXP = 100          # padded context rows: org 0:77, zeros 77:96, ipa 96:100
BIASROW = 95        # pad row used to carry the output bias
NSH = 2048          # query rows per core
KBQ = QD // P       # 5
KBC = CD // P       # 6
KBA = 2 * KBC       # 12 (augmented contraction blocks)
NCH = NSH // 512    # 4 chunks of 512 queries
CH = 512

F32 = mybir.dt.float32
F32R = mybir.dt.float32r
BF16 = mybir.dt.bfloat16

_NC_CACHE = None


def build_nc():
    nc = bacc.Bacc(None, target_bir_lowering=False, debug=False)

    x = nc.declare_dram_parameter("x", [NSH, QD], F32, isOutput=False)
    ctx = nc.declare_dram_parameter("ctx", [81, CD], F32, isOutput=False)
    wqt = nc.declare_dram_parameter("wqt", [DH, H, QD], F32, isOutput=False)
    wkc = nc.declare_dram_parameter("wkc", [P, 2, KBA, 320], F32, isOutput=False)
    wvc = nc.declare_dram_parameter("wvc", [P, 2, KBA, 320], F32, isOutput=False)
    wo = nc.declare_dram_parameter("wo", [DH, H, QD], F32, isOutput=False)
    boutb = nc.declare_dram_parameter("boutb", [QD], BF16, isOutput=False)
    sel16 = nc.declare_dram_parameter("sel16", [CTXP, H, 16], BF16, isOutput=False)
    sel2t = nc.declare_dram_parameter("sel2t", [17, H, CTXP], F32, isOutput=False)
    identd = nc.declare_dram_parameter("identd", [P, P], F32, isOutput=False)
    zaug = nc.declare_dram_parameter("zaug", [P, KBA * CTXP], F32, isOutput=False)
    out = nc.declare_dram_parameter("out", [NSH, QD], BF16, isOutput=True)

    Exp = mybir.ActivationFunctionType.Exp
    Copy = mybir.ActivationFunctionType.Copy

    def evict(eng, dst, src):
        if eng == "act":
            nc.scalar.activation(dst, src, Copy)
        else:
            nc.vector.tensor_copy(dst, src)

    with tile.TileContext(nc) as tc:
        with (
            tc.tile_pool(name="const", bufs=1) as const,
            tc.tile_pool(name="persist", bufs=1) as persist,
            tc.tile_pool(name="wbig", bufs=2) as wbig,
            tc.tile_pool(name="wqo", bufs=1) as wqo,
            tc.tile_pool(name="xtp", bufs=2) as xtp,
            tc.tile_pool(name="esp", bufs=9) as esp,
            tc.tile_pool(name="atp", bufs=9) as atp,
            tc.tile_pool(name="rsp", bufs=2) as rsp,
            tc.tile_pool(name="xs", bufs=6) as xs,
            tc.tile_pool(name="ys", bufs=3) as ys,
            tc.tile_pool(name="ps", bufs=7, space="PSUM") as ps,
            tc.tile_pool(name="psum_sums", bufs=1, space="PSUM") as ps_sums,
        ):
            # ---- constants (DMA'd; identity comes from the host) ----
            ident32 = const.tile([P, P], F32)
            nc.scalar.dma_start(out=ident32, in_=identd[:, :])
            ident = const.tile([P, P], F32R)
            nc.scalar.dma_start(out=ident, in_=identd[:, :].bitcast(F32R))
            # ---- all large DMAs issued up front. Weights stream on the
            #      sync hw queue in critical-path order; x0 + constants ride
            #      the scalar hw queue. ----
            ctx_sb = persist.tile([81, CD], F32)
            nc.sync.dma_start(out=ctx_sb, in_=ctx[:, :])
            x0_tiles = []
            for jj in range(4):
                x_t0 = xs.tile([P, QD], F32R, tag="xs")
                nc.scalar.dma_start(out=x_t0,
                                    in_=x[jj * P:(jj + 1) * P, :].bitcast(F32R))
                x0_tiles.append(x_t0)
            ctxT_aug = persist.tile([P, KBA * CTXP], F32R)
            nc.scalar.dma_start(out=ctxT_aug, in_=zaug[:, :].bitcast(F32R))
            sel16_sb = const.tile([CTXP, H, 16], BF16)
            nc.scalar.dma_start(out=sel16_sb, in_=sel16[:, :, :])
            sel2t_sb = const.tile([17, H, CTXP], F32R)
            nc.scalar.dma_start(out=sel2t_sb, in_=sel2t[:, :, :].bitcast(F32R))
            # weight tiles allocate in ring order wkc, wvc, wo (wo reuses
            # wkc's slot once the k projection has consumed it)
            wkc_sb = wbig.tile([P, 2, KBA, 320], F32R, tag="wbig")
            wvc_sb = wbig.tile([P, 2, KBA, 320], F32R, tag="wbig")
            wo_sb = wbig.tile([DH, H, QD], F32R, tag="wbig")
            wqt_sb = wqo.tile([DH, H, QD], F32R, tag="wqo")
            nc.sync.dma_start(out=wkc_sb[:, 0, 0:6], in_=wkc[:, 0, 0:6].bitcast(F32R))
            nc.sync.dma_start(out=wkc_sb[:, 0, 6:12], in_=wkc[:, 0, 6:12].bitcast(F32R))
            nc.sync.dma_start(out=wqt_sb[:, 0:4, :], in_=wqt[:, 0:4, :].bitcast(F32R))
            nc.sync.dma_start(out=wkc_sb[:, 1, 0:6], in_=wkc[:, 1, 0:6].bitcast(F32R))
            nc.sync.dma_start(out=wkc_sb[:, 1, 6:12], in_=wkc[:, 1, 6:12].bitcast(F32R))
            nc.sync.dma_start(out=wqt_sb[:, 4:8, :], in_=wqt[:, 4:8, :].bitcast(F32R))
            nc.sync.dma_start(out=wo_sb, in_=wo[:, :, :].bitcast(F32R))
            nc.sync.dma_start(out=wvc_sb[:, 0], in_=wvc[:, 0].bitcast(F32R))
            nc.sync.dma_start(out=wvc_sb[:, 1], in_=wvc[:, 1].bitcast(F32R))

            # ---- context: transpose into augmented layout ----
            for kb in range(KBC):
                pc = ps.tile([P, 81], F32, tag="ps")
                nc.tensor.transpose(pc, ctx_sb[:, kb * P:(kb + 1) * P],
                                    ident32[0:81, 0:81])
                eng = "act" if kb % 2 == 0 else "dve"
                evict(eng, ctxT_aug[:, kb * CTXP:kb * CTXP + ORG],
                      pc[:, 0:ORG])
                evict(eng, ctxT_aug[:, (kb + KBC) * CTXP + 96:
                                    (kb + KBC + 1) * CTXP],
                      pc[:, ORG:81])

            # ---- x transpose (emitted before the k projection so the PE has
            #      work while the weight DMAs stream in) ----
            def emit_x_dma(c):
                tiles = []
                for jj in range(4):
                    j = c * 4 + jj
                    x_t = xs.tile([P, QD], F32R, tag="xs")
                    nc.scalar.dma_start(
                        out=x_t, in_=x[j * P:(j + 1) * P, :].bitcast(F32R))
                    tiles.append(x_t)
                return tiles

            def emit_xpose(c, x_tiles):
                xT_c = xtp.tile([P, KBQ, CH], F32R, tag="xt")
                for jj in range(4):
                    x_t = x_tiles[jj]
                    pA = ps.tile([P, CH], F32R, tag="ps")
                    pB = ps.tile([P, P], F32R, tag="ps")
                    for p in range(4):
                        nc.tensor.transpose(pA[:, p * P:(p + 1) * P],
                                            x_t[:, p * P:(p + 1) * P], ident)
                    nc.tensor.transpose(pB, x_t[:, 512:640], ident)
                    evict("act", xT_c[:, 0:4, jj * P:(jj + 1) * P],
                          pA.rearrange("p (b q) -> p b q", b=4))
                    evict("act", xT_c[:, 4, jj * P:(jj + 1) * P], pB)
                return xT_c

            xT_cur = emit_xpose(0, x0_tiles)

            # ---- k projection, kT, M in head-halves so sims h<4 can start
            #      while the second half of the K weights stream ----
            ksb = persist.tile([CTXP, INNER], F32R)
            kT = persist.tile([DH, H, CTXP], F32R)
            M_sb = persist.tile([P, KBQ, H, CTXP], F32R)

            def emit_kchain(chh):
                sl = slice(chh * 320, (chh + 1) * 320)
                pk = ps.tile([CTXP, 320], F32, tag="ps")
                for kb in range(KBA):
                    nc.tensor.matmul(pk,
                                     ctxT_aug[:, kb * CTXP:(kb + 1) * CTXP],
                                     wkc_sb[:, chh, kb, :],
                                     start=(kb == 0), stop=(kb == KBA - 1))
                evict("act" if chh == 0 else "dve", ksb[:, sl], pk)
                for h in range(chh * 4, chh * 4 + 4):
                    pt = ps.tile([DH, CTXP], F32R, tag="ps")
                    nc.tensor.transpose(pt, ksb[:, h * DH:(h + 1) * DH],
                                        ident[0:CTXP, 0:CTXP])
                    evict("dve" if h % 2 == 0 else "act", kT[:, h, :], pt)
                for h in range(chh * 4, chh * 4 + 4):
                    pm = ps.tile([P, 500], F32, tag="ps")
                    for kb in range(KBQ):
                        nc.tensor.matmul(pm[:, kb * CTXP:(kb + 1) * CTXP],
                                         wqt_sb[:, h, kb * P:(kb + 1) * P],
                                         kT[:, h, :], start=True, stop=True)
                    evict("act" if h % 2 == 0 else "dve",
                          M_sb[:, :, h, :],
                          pm.rearrange("p (kb c) -> p kb c", kb=KBQ))

            emit_kchain(0)

            VW_sb = persist.tile([CTXP, H, INNER], BF16)
            vsb = persist.tile([CTXP, INNER], F32R)
            vT = persist.tile([DH, H, CTXP], F32R)
            # bias row of VW (evictions skip row 95, so no WAW dependency)
            nc.scalar.dma_start(
                out=VW_sb[BIASROW:BIASROW + 1, 0, :],
                in_=bass.AP(tensor=boutb, offset=0, ap=[[0, 1], [1, QD]]))

            def emit_vchain():
                # v projection + vT + VW_h = v_h @ Wout_h; emitted inside
                # chunk 0 so the PE isn't queued behind the wvc/wo DMAs.
                for chh in range(2):
                    sl = slice(chh * 320, (chh + 1) * 320)
                    pv = ps.tile([CTXP, 320], F32, tag="ps")
                    for kb in range(KBA):
                        nc.tensor.matmul(pv,
                                         ctxT_aug[:, kb * CTXP:(kb + 1) * CTXP],
                                         wvc_sb[:, chh, kb, :],
                                         start=(kb == 0), stop=(kb == KBA - 1))
                    evict("dve" if chh == 0 else "act", vsb[:, sl], pv)
                for h in range(H):
                    pt2 = ps.tile([DH, CTXP], F32R, tag="ps")
                    nc.tensor.transpose(pt2, vsb[:, h * DH:(h + 1) * DH],
                                        ident[0:CTXP, 0:CTXP])
                    evict("act" if h % 2 == 0 else "dve", vT[:, h, :], pt2)
                for h in range(H):
                    pw1 = ps.tile([CTXP, CH], F32, tag="ps")
                    nc.tensor.matmul(pw1, vT[:, h, :], wo_sb[:, h, 0:512],
                                     start=True, stop=True)
                    pw2 = ps.tile([CTXP, P], F32, tag="ps")
                    nc.tensor.matmul(pw2, vT[:, h, :], wo_sb[:, h, 512:640],
                                     start=True, stop=True)
                    eng = "act" if h % 2 == 0 else "dve"
                    if h == 0:
                        # skip the bias row so the boutb DMA has no WAW dep
                        evict(eng, VW_sb[0:BIASROW, h, 0:512],
                              pw1[0:BIASROW, :])
                        evict(eng, VW_sb[96:CTXP, h, 0:512], pw1[96:CTXP, :])
                        evict(eng, VW_sb[0:BIASROW, h, 512:640],
                              pw2[0:BIASROW, :])
                        evict(eng, VW_sb[96:CTXP, h, 512:640], pw2[96:CTXP, :])
                    else:
                        evict(eng, VW_sb[:, h, 0:512], pw1)
                        evict(eng, VW_sb[:, h, 512:640], pw2)

            # ---- main per-chunk pipeline ----
            def emit_yproj_j(at_tiles, c, jj):
                j = c * 4 + jj
                jsl = slice(jj * P, (jj + 1) * P)
                pyA = ps.tile([P, CH], F32, tag="ps")
                for h in range(H):
                    nc.tensor.matmul(pyA, at_tiles[h][:, jsl],
                                     VW_sb[:, h, 0:512],
                                     start=(h == 0), stop=(h == H - 1))
                pyB = ps.tile([P, P], F32, tag="ps")
                for h in range(H):
                    nc.tensor.matmul(pyB, at_tiles[h][:, jsl],
                                     VW_sb[:, h, 512:640],
                                     start=(h == 0), stop=(h == H - 1))
                y_t = ys.tile([P, QD], BF16, tag="ys")
                evict("act" if jj % 2 == 0 else "dve", y_t[:, 0:512], pyA)
                evict("dve" if jj % 2 == 0 else "act", y_t[:, 512:640], pyB)
                nc.sync.dma_start(out=out[j * P:(j + 1) * P, :], in_=y_t)

            pending = None  # previous chunk's deferred j=3 tile
            for c in range(NCH):
                # sim + exp per head
                es_tiles = []
                for h in range(H):
                    if c == 0 and h == 4:
                        emit_kchain(1)
                    psim = ps.tile([CTXP, CH], F32, tag="ps")
                    for kb in range(KBQ):
                        nc.tensor.matmul(psim, M_sb[:, kb, h, :],
                                         xT_cur[:, kb, :],
                                         start=(kb == 0), stop=(kb == KBQ - 1))
                    es_t = esp.tile([CTXP, CH], BF16, tag="es")
                    nc.scalar.activation(es_t, psim, Exp)
                    es_tiles.append(es_t)
                # per-(head,range) sums accumulated into one bank
                sums16 = ps_sums.tile([16, CH], F32, tag="sums")
                for h in range(H):
                    nc.tensor.matmul(sums16, sel16_sb[:, h, :], es_tiles[h],
                                     start=(h == 0), stop=(h == H - 1),
                                     skip_group_check=True)
                if c == 0:
                    emit_vchain()
                # PE fillers for the recip serial chain: next chunk's x
                # transpose + the previous chunk's deferred yproj tile
                xT_next = (emit_xpose(c + 1, emit_x_dma(c + 1))
                           if c + 1 < NCH else None)
                if pending is not None:
                    emit_yproj_j(pending, c - 1, 3)
                # one wide reciprocal per chunk; row 16 == 1.0 for the bias row
                rs_t = rsp.tile([17, CH], F32, tag="rs")
                nc.vector.memset(rs_t, 1.0)
                nc.vector.reciprocal_approx_fast(rs_t[0:16, :], sums16)
                rs_r = rsp.tile([17, CH], F32R, tag="rsr")
                nc.vector.tensor_copy(rs_r, rs_t)
                # broadcast + normalize -> bf16 attention weights
                at_tiles = []
                for h in range(H):
                    pbrd = ps.tile([CTXP, CH], F32, tag="ps")
                    nc.tensor.matmul(pbrd, sel2t_sb[:, h, :], rs_r,
                                     start=True, stop=True)
                    at_t = atp.tile([CTXP, CH], BF16, tag="at")
                    nc.vector.tensor_mul(at_t, es_tiles[h], pbrd)
                    at_tiles.append(at_t)
                # y projection j=0..2 now; j=3 deferred into the next chunk
                for jj in range(3):
                    emit_yproj_j(at_tiles, c, jj)
                pending = at_tiles
                xT_cur = xT_next
            emit_yproj_j(pending, NCH - 1, 3)

    nc.finalize()
    return nc


def _get_nc():
    global _NC_CACHE
    if _NC_CACHE is None:
        _NC_CACHE = build_nc()
    return _NC_CACHE


def _fold_weights(inputs):
    f = lambda k: np.asarray(inputs[k], np.float64)
    scale = DH ** -0.5
    wq = (f("Wq") + f("q_down") @ f("q_up") * (float(inputs["q_alpha"]) / 16.0)) * scale
    wk = f("Wk") + f("k_down") @ f("k_up") * (float(inputs["k_alpha"]) / 16.0)
    wv = f("Wv") + f("v_down") @ f("v_up") * (float(inputs["v_alpha"]) / 16.0)
    wo = f("Wout") + f("o_down") @ f("o_up") * (float(inputs["o_alpha"]) / 16.0)
    return (wq.astype(np.float32), wk.astype(np.float32), wv.astype(np.float32),
            wo.astype(np.float32))


def kernel(trace=False, **inputs):
    nc = _get_nc()
    x = np.ascontiguousarray(np.asarray(inputs["x"], np.float32))
    context = np.ascontiguousarray(np.asarray(inputs["context"], np.float32))
    wq, wk, wv, wo = _fold_weights(inputs)
    wki = np.asarray(inputs["Wk_ipa"], np.float32)
    wvi = np.asarray(inputs["Wv_ipa"], np.float32)
    bout = np.asarray(inputs["bout"], np.float32)

    # wqT head-major [DH, H, QD]
    wqt = np.ascontiguousarray(wq.T.reshape(H, DH, QD).transpose(1, 0, 2))
    # wo head-major [DH, H, QD]
    wo_hm = np.ascontiguousarray(wo.reshape(H, DH, QD).transpose(1, 0, 2))
    def chmajor(w):
        # [1536, 640] -> [P, 2, KBA, 320] (partition-major, col-chunk-major)
        return np.ascontiguousarray(
            w.reshape(KBA, P, 2, 320).transpose(1, 2, 0, 3))
    wkc = chmajor(np.concatenate([wk, wki], axis=0))
    wvc = chmajor(np.concatenate([wv, wvi], axis=0))
    boutb = bout.astype(ml_dtypes.bfloat16)

    sel16_h = np.zeros((CTXP, H, 16), np.float32)
    sel2t_h = np.zeros((17, H, CTXP), np.float32)
    for h in range(H):
        sel16_h[0:ORG, h, 2 * h] = 1.0
        sel16_h[96:CTXP, h, 2 * h + 1] = 1.0
        sel2t_h[2 * h, h, 0:ORG] = 1.0
        sel2t_h[2 * h + 1, h, 96:CTXP] = 1.0
    sel2t_h[16, 0, BIASROW] = 1.0
    sel16_b = sel16_h.astype(ml_dtypes.bfloat16)

    shared = dict(wqt=wqt, wkc=wkc, wvc=wvc, wo=wo_hm, boutb=boutb,
                  sel16=sel16_b, sel2t=sel2t_h,
                  identd=np.eye(P, dtype=np.float32),
                  zaug=np.zeros((P, KBA * CTXP), np.float32))
    in_maps = []
    for i in range(8):
        b, half = i // 2, i % 2
        in_maps.append(dict(
            x=np.ascontiguousarray(x[b, half * NSH:(half + 1) * NSH, :]),
            ctx=np.ascontiguousarray(context[b]),
            **shared,
        ))
    res = run_bass_kernel_spmd(nc, in_maps, list(range(8)), trace=trace)
    outp = np.empty((B, N, QD), np.float32)
    for i in range(8):
        b, half = i // 2, i % 2
        outp[b, half * NSH:(half + 1) * NSH, :] = np.asarray(
            res.results[i]["out"]).astype(np.float32)
    if trace:
        return outp, res
    return outp


# revision 49
# speedup vs baseline: 1.0584x; 1.0584x over previous
"""Fused cross-attention (LoRA + IP-Adapter) Trainium2 kernel, v2.

Sharding: data-parallel over (batch, seq/2) -> 8 shards of 2048 query rows.
Each core computes its shard end-to-end; no collectives. LoRA deltas and the
attention scale are folded into the projection weights on the host.

v2 design (vs v1): weight-folding on device to shrink the per-chunk loop.
  - M_h = WqT_h @ kT_h  [640, 100] per head, computed once. Then
    simT_h = M_h^T @ xT directly (5 accum matmuls) -- no per-chunk q
    projection or qT eviction.
  - VW_h = v_h @ Wout_h [100, 640] per head (bf16), computed once. Then
    y_j = sum_h at_h[:, j]^T @ VW_h -- the normalized attention weights are
    the stationary operand; no AV matmul, no outT buffer.
  - Softmax: exp -> per-(head,range) sums accumulated into one [16,512]
    PSUM tile via per-head selector matmuls -> a single
    reciprocal_approx_fast per 512-query chunk -> per-head broadcast
    matmul -> one DVE multiply producing bf16 attention weights.
  - Bias: pad context row 95 carries at==1 (selector row 16) and
    VW[95, h=0, :] = bout, so yproj accumulates the bias for free.
  - Context is embedded into an augmented [1536, 640] contraction
    (org tokens use rows 0:768 = Wk/Wv, IPA tokens rows 768:1536 =
    Wk_ipa/Wv_ipa), so one matmul pair projects k and v for all tokens.
"""
import sys, types

for _p in ("/opt/trn_rl_repo", "/root/.axon_site", "/root/.axon_site/_ro/trn_rl_repo"):
    if _p not in sys.path:
        sys.path.append(_p)

import numpy as np


def install_ntff_shim():
    """The image's antenv lacks axon_hooks; inject it and register the ctypes
    NTFF profile hook so run_bass_kernel_spmd(trace=True) yields exec_time_ns."""
    if "antenv.axon_hooks" in sys.modules:
        return
    mod = types.ModuleType("antenv.axon_hooks")
    mod._hook = None

    def set_axon_ntff_profile_hook(hook):
        mod._hook = hook

    def get_axon_ntff_profile_hook():
        return mod._hook

    mod.set_axon_ntff_profile_hook = set_axon_ntff_profile_hook
    mod.get_axon_ntff_profile_hook = get_axon_ntff_profile_hook
    sys.modules["antenv.axon_hooks"] = mod
    try:
        from trn_agent_boot.trn_boot import _ntff_profile_via_ctypes
        mod._hook = _ntff_profile_via_ctypes("/opt/axon/libaxon_pjrt.so")
    except Exception:
        pass


install_ntff_shim()

import ml_dtypes
import concourse.bass as bass
import concourse.bacc as bacc
import concourse.tile as tile
from concourse import mybir
from concourse.bass_utils import run_bass_kernel_spmd

P = 128
B, N, QD, CD = 4, 4096, 640, 768
H, DH = 8, 80
INNER = 640
ORG, IPA = 77, 4
CTXP = 100          # padded context rows: org 0:77, zeros 77:96, ipa 96:100
BIASROW = 95        # pad row used to carry the output bias
NSH = 2048          # query rows per core
KBQ = QD // P       # 5
KBC = CD // P       # 6
KBA = 2 * KBC       # 12 (augmented contraction blocks)
NCH = NSH // 512    # 4 chunks of 512 queries
CH = 512

F32 = mybir.dt.float32
F32R = mybir.dt.float32r
BF16 = mybir.dt.bfloat16

_NC_CACHE = None


def build_nc():
    nc = bacc.Bacc(None, target_bir_lowering=False, debug=False)

    x = nc.declare_dram_parameter("x", [NSH, QD], F32, isOutput=False)
    ctx = nc.declare_dram_parameter("ctx", [81, CD], F32, isOutput=False)
    wqt = nc.declare_dram_parameter("wqt", [DH, H, QD], F32, isOutput=False)
    wkc = nc.declare_dram_parameter("wkc", [P, 2, KBA, 320], F32, isOutput=False)
    wvc = nc.declare_dram_parameter("wvc", [P, 2, KBA, 320], F32, isOutput=False)
    wo = nc.declare_dram_parameter("wo", [DH, H, QD], F32, isOutput=False)
    boutb = nc.declare_dram_parameter("boutb", [QD], BF16, isOutput=False)
    sel16 = nc.declare_dram_parameter("sel16", [CTXP, H, 16], BF16, isOutput=False)
    sel2t = nc.declare_dram_parameter("sel2t", [17, H, CTXP], F32, isOutput=False)
    identd = nc.declare_dram_parameter("identd", [P, P], F32, isOutput=False)
    zaug = nc.declare_dram_parameter("zaug", [P, KBA * CTXP], F32, isOutput=False)
    out = nc.declare_dram_parameter("out", [NSH, QD], BF16, isOutput=True)

    Exp = mybir.ActivationFunctionType.Exp
    Copy = mybir.ActivationFunctionType.Copy

    def evict(eng, dst, src):
        if eng == "act":
            nc.scalar.activation(dst, src, Copy)
        else:
            nc.vector.tensor_copy(dst, src)

    with tile.TileContext(nc) as tc:
        with (
            tc.tile_pool(name="const", bufs=1) as const,
            tc.tile_pool(name="persist", bufs=1) as persist,
            tc.tile_pool(name="wbig", bufs=2) as wbig,
            tc.tile_pool(name="wqo", bufs=1) as wqo,
            tc.tile_pool(name="xtp", bufs=2) as xtp,
            tc.tile_pool(name="esp", bufs=9) as esp,
            tc.tile_pool(name="atp", bufs=9) as atp,
            tc.tile_pool(name="rsp", bufs=2) as rsp,
            tc.tile_pool(name="xs", bufs=6) as xs,
            tc.tile_pool(name="ys", bufs=3) as ys,
            tc.tile_pool(name="ps", bufs=7, space="PSUM") as ps,
            tc.tile_pool(name="psum_sums", bufs=1, space="PSUM") as ps_sums,
        ):
            # ---- constants (DMA'd; identity comes from the host) ----
            ident32 = const.tile([P, P], F32)
            nc.scalar.dma_start(out=ident32, in_=identd[:, :])
            ident = const.tile([P, P], F32R)
            nc.scalar.dma_start(out=ident, in_=identd[:, :].bitcast(F32R))
            # ---- all large DMAs issued up front. Weights stream on the
            #      sync hw queue in critical-path order; x0 + constants ride
            #      the scalar hw queue. ----
            ctx_sb = persist.tile([81, CD], F32)
            nc.sync.dma_start(out=ctx_sb, in_=ctx[:, :])
            x0_tiles = []
            for jj in range(4):
                x_t0 = xs.tile([P, QD], F32R, tag="xs")
                nc.scalar.dma_start(out=x_t0,
                                    in_=x[jj * P:(jj + 1) * P, :].bitcast(F32R))
                x0_tiles.append(x_t0)
            ctxT_aug = persist.tile([P, KBA * CTXP], F32R)
            nc.scalar.dma_start(out=ctxT_aug, in_=zaug[:, :].bitcast(F32R))
            sel16_sb = const.tile([CTXP, H, 16], BF16)
            nc.scalar.dma_start(out=sel16_sb, in_=sel16[:, :, :])
            sel2t_sb = const.tile([17, H, CTXP], F32R)
            nc.scalar.dma_start(out=sel2t_sb, in_=sel2t[:, :, :].bitcast(F32R))
            # weight tiles allocate in ring order wkc, wvc, wo (wo reuses
            # wkc's slot once the k projection has consumed it)
            wkc_sb = wbig.tile([P, 2, KBA, 320], F32R, tag="wbig")
            wvc_sb = wbig.tile([P, 2, KBA, 320], F32R, tag="wbig")
            wo_sb = wbig.tile([DH, H, QD], F32R, tag="wbig")
            wqt_sb = wqo.tile([DH, H, QD], F32R, tag="wqo")
            nc.sync.dma_start(out=wkc_sb[:, 0, 0:6], in_=wkc[:, 0, 0:6].bitcast(F32R))
            nc.sync.dma_start(out=wkc_sb[:, 0, 6:12], in_=wkc[:, 0, 6:12].bitcast(F32R))
            nc.sync.dma_start(out=wqt_sb[:, 0:4, :], in_=wqt[:, 0:4, :].bitcast(F32R))
            nc.sync.dma_start(out=wkc_sb[:, 1, 0:6], in_=wkc[:, 1, 0:6].bitcast(F32R))
            nc.sync.dma_start(out=wkc_sb[:, 1, 6:12], in_=wkc[:, 1, 6:12].bitcast(F32R))
            nc.sync.dma_start(out=wqt_sb[:, 4:8, :], in_=wqt[:, 4:8, :].bitcast(F32R))
            nc.sync.dma_start(out=wo_sb, in_=wo[:, :, :].bitcast(F32R))
            nc.sync.dma_start(out=wvc_sb[:, 0], in_=wvc[:, 0].bitcast(F32R))
            nc.sync.dma_start(out=wvc_sb[:, 1], in_=wvc[:, 1].bitcast(F32R))

            # ---- context: transpose into augmented layout ----
            for kb in range(KBC):
                pc = ps.tile([P, 81], F32, tag="ps")
                nc.tensor.transpose(pc, ctx_sb[:, kb * P:(kb + 1) * P],
                                    ident32[0:81, 0:81])
                eng = "act" if kb % 2 == 0 else "dve"
                evict(eng, ctxT_aug[:, kb * CTXP:kb * CTXP + ORG],
                      pc[:, 0:ORG])
                evict(eng, ctxT_aug[:, (kb + KBC) * CTXP + 96:
                                    (kb + KBC + 1) * CTXP],
                      pc[:, ORG:81])

            # ---- x transpose (emitted before the k projection so the PE has
            #      work while the weight DMAs stream in) ----
            def emit_x_dma(c):
                tiles = []
                for jj in range(4):
                    j = c * 4 + jj
                    x_t = xs.tile([P, QD], F32R, tag="xs")
                    nc.scalar.dma_start(
                        out=x_t, in_=x[j * P:(j + 1) * P, :].bitcast(F32R))
                    tiles.append(x_t)
                return tiles

            def emit_xpose(c, x_tiles):
                xT_c = xtp.tile([P, KBQ, CH], F32R, tag="xt")
                for jj in range(4):
                    x_t = x_tiles[jj]
                    pA = ps.tile([P, CH], F32R, tag="ps")
                    pB = ps.tile([P, P], F32R, tag="ps")
                    for p in range(4):
                        nc.tensor.transpose(pA[:, p * P:(p + 1) * P],
                                            x_t[:, p * P:(p + 1) * P], ident)
                    nc.tensor.transpose(pB, x_t[:, 512:640], ident)
                    evict("act", xT_c[:, 0:4, jj * P:(jj + 1) * P],
                          pA.rearrange("p (b q) -> p b q", b=4))
                    evict("act", xT_c[:, 4, jj * P:(jj + 1) * P], pB)
                return xT_c

            xT_cur = emit_xpose(0, x0_tiles)

            # ---- k projection, kT, M in head-halves so sims h<4 can start
            #      while the second half of the K weights stream ----
            ksb = persist.tile([CTXP, INNER], F32R)
            kT = persist.tile([DH, H, CTXP], F32R)
            M_sb = persist.tile([P, KBQ, H, CTXP], F32R)

            def emit_kchain(chh):
                sl = slice(chh * 320, (chh + 1) * 320)
                pk = ps.tile([CTXP, 320], F32, tag="ps")
                for kb in range(KBA):
                    nc.tensor.matmul(pk,
                                     ctxT_aug[:, kb * CTXP:(kb + 1) * CTXP],
                                     wkc_sb[:, chh, kb, :],
                                     start=(kb == 0), stop=(kb == KBA - 1))
                evict("act" if chh == 0 else "dve", ksb[:, sl], pk)
                for h in range(chh * 4, chh * 4 + 4):
                    pt = ps.tile([DH, CTXP], F32R, tag="ps")
                    nc.tensor.transpose(pt, ksb[:, h * DH:(h + 1) * DH],
                                        ident[0:CTXP, 0:CTXP])
                    evict("dve" if h % 2 == 0 else "act", kT[:, h, :], pt)
                for h in range(chh * 4, chh * 4 + 4):
                    pm = ps.tile([P, 500], F32, tag="ps")
                    for kb in range(KBQ):
                        nc.tensor.matmul(pm[:, kb * CTXP:(kb + 1) * CTXP],
                                         wqt_sb[:, h, kb * P:(kb + 1) * P],
                                         kT[:, h, :], start=True, stop=True)
                    evict("act" if h % 2 == 0 else "dve",
                          M_sb[:, :, h, :],
                          pm.rearrange("p (kb c) -> p kb c", kb=KBQ))

            emit_kchain(0)

            VW_sb = persist.tile([CTXP, H, INNER], BF16)
            vsb = persist.tile([CTXP, INNER], F32R)
            vT = persist.tile([DH, H, CTXP], F32R)
            # bias row of VW (evictions skip row 95, so no WAW dependency)
            nc.scalar.dma_start(
                out=VW_sb[BIASROW:BIASROW + 1, 0, :],
                in_=bass.AP(tensor=boutb, offset=0, ap=[[0, 1], [1, QD]]))

            def emit_vchain():
                # v projection + vT + VW_h = v_h @ Wout_h; emitted inside
                # chunk 0 so the PE isn't queued behind the wvc/wo DMAs.
                for chh in range(2):
                    sl = slice(chh * 320, (chh + 1) * 320)
                    pv = ps.tile([CTXP, 320], F32, tag="ps")
                    for kb in range(KBA):
                        nc.tensor.matmul(pv,
                                         ctxT_aug[:, kb * CTXP:(kb + 1) * CTXP],
                                         wvc_sb[:, chh, kb, :],
                                         start=(kb == 0), stop=(kb == KBA - 1))
                    evict("dve" if chh == 0 else "act", vsb[:, sl], pv)
                for h in range(H):
                    pt2 = ps.tile([DH, CTXP], F32R, tag="ps")
                    nc.tensor.transpose(pt2, vsb[:, h * DH:(h + 1) * DH],
                                        ident[0:CTXP, 0:CTXP])
                    evict("act" if h % 2 == 0 else "dve", vT[:, h, :], pt2)
                for h in range(H):
                    pw1 = ps.tile([CTXP, CH], F32, tag="ps")
                    nc.tensor.matmul(pw1, vT[:, h, :], wo_sb[:, h, 0:512],
                                     start=True, stop=True)
                    pw2 = ps.tile([CTXP, P], F32, tag="ps")
                    nc.tensor.matmul(pw2, vT[:, h, :], wo_sb[:, h, 512:640],
                                     start=True, stop=True)
                    eng = "act" if h % 2 == 0 else "dve"
                    if h == 0:
                        # skip the bias row so the boutb DMA has no WAW dep
                        evict(eng, VW_sb[0:BIASROW, h, 0:512],
                              pw1[0:BIASROW, :])
                        evict(eng, VW_sb[96:CTXP, h, 0:512], pw1[96:CTXP, :])
                        evict(eng, VW_sb[0:BIASROW, h, 512:640],
                              pw2[0:BIASROW, :])
                        evict(eng, VW_sb[96:CTXP, h, 512:640], pw2[96:CTXP, :])
                    else:
                        evict(eng, VW_sb[:, h, 0:512], pw1)
                        evict(eng, VW_sb[:, h, 512:640], pw2)

            # ---- main per-chunk pipeline ----
            def emit_yproj_j(at_tiles, c, jj):
                j = c * 4 + jj
                jsl = slice(jj * P, (jj + 1) * P)
                pyA = ps.tile([P, CH], F32, tag="ps")
                for h in range(H):
                    nc.tensor.matmul(pyA, at_tiles[h][:, jsl],
                                     VW_sb[:, h, 0:512],
                                     start=(h == 0), stop=(h == H - 1))
                pyB = ps.tile([P, P], F32, tag="ps")
                for h in range(H):
                    nc.tensor.matmul(pyB, at_tiles[h][:, jsl],
                                     VW_sb[:, h, 512:640],
                                     start=(h == 0), stop=(h == H - 1))
                y_t = ys.tile([P, QD], BF16, tag="ys")
                evict("act" if jj % 2 == 0 else "dve", y_t[:, 0:512], pyA)
                evict("dve" if jj % 2 == 0 else "act", y_t[:, 512:640], pyB)
                nc.sync.dma_start(out=out[j * P:(j + 1) * P, :], in_=y_t)

            pending = []  # deferred (at_tiles, chunk, j) yproj items
            for c in range(NCH):
                # sim + exp per head
                es_tiles = []
                for h in range(H):
                    if c == 0 and h == 4:
                        emit_kchain(1)
                    psim = ps.tile([CTXP, CH], F32, tag="ps")
                    for kb in range(KBQ):
                        nc.tensor.matmul(psim, M_sb[:, kb, h, :],
                                         xT_cur[:, kb, :],
                                         start=(kb == 0), stop=(kb == KBQ - 1))
                    es_t = esp.tile([CTXP, CH], BF16, tag="es")
                    nc.scalar.activation(es_t, psim, Exp)
                    es_tiles.append(es_t)
                # per-(head,range) sums accumulated into one bank
                sums16 = ps_sums.tile([16, CH], F32, tag="sums")
                for h in range(H):
                    nc.tensor.matmul(sums16, sel16_sb[:, h, :], es_tiles[h],
                                     start=(h == 0), stop=(h == H - 1),
                                     skip_group_check=True)
                # PE fillers for the recip serial chain: next chunk's x
                # transpose + the previous chunk's deferred yproj tiles
                xT_next = (emit_xpose(c + 1, emit_x_dma(c + 1))
                           if c + 1 < NCH else None)
                for (p_at, p_c, p_jj) in pending:
                    emit_yproj_j(p_at, p_c, p_jj)
                pending = []
                # one wide reciprocal per chunk; row 16 == 1.0 for the bias row
                rs_t = rsp.tile([17, CH], F32, tag="rs")
                nc.vector.memset(rs_t, 1.0)
                nc.vector.reciprocal_approx_fast(rs_t[0:16, :], sums16)
                rs_r = rsp.tile([17, CH], F32R, tag="rsr")
                nc.vector.tensor_copy(rs_r, rs_t)
                # broadcast + normalize -> bf16 attention weights
                at_tiles = []
                for h in range(H):
                    pbrd = ps.tile([CTXP, CH], F32, tag="ps")
                    nc.tensor.matmul(pbrd, sel2t_sb[:, h, :], rs_r,
                                     start=True, stop=True)
                    at_t = atp.tile([CTXP, CH], BF16, tag="at")
                    nc.vector.tensor_mul(at_t, es_tiles[h], pbrd)
                    at_tiles.append(at_t)
                if c == 0:
                    # v-chain here: wvc/wo have arrived by now, and chunk 0's
                    # whole yproj is deferred into chunk 1 (VW ready there)
                    emit_vchain()
                    pending = [(at_tiles, 0, jj) for jj in range(4)]
                else:
                    # y projection j=0..2 now; j=3 deferred into next chunk
                    for jj in range(3):
                        emit_yproj_j(at_tiles, c, jj)
                    pending = [(at_tiles, c, 3)]
                xT_cur = xT_next
            for (p_at, p_c, p_jj) in pending:
                emit_yproj_j(p_at, p_c, p_jj)

    nc.finalize()
    return nc


def _get_nc():
    global _NC_CACHE
    if _NC_CACHE is None:
        _NC_CACHE = build_nc()
    return _NC_CACHE


def _fold_weights(inputs):
    f = lambda k: np.asarray(inputs[k], np.float64)
    scale = DH ** -0.5
    wq = (f("Wq") + f("q_down") @ f("q_up") * (float(inputs["q_alpha"]) / 16.0)) * scale
    wk = f("Wk") + f("k_down") @ f("k_up") * (float(inputs["k_alpha"]) / 16.0)
    wv = f("Wv") + f("v_down") @ f("v_up") * (float(inputs["v_alpha"]) / 16.0)
    wo = f("Wout") + f("o_down") @ f("o_up") * (float(inputs["o_alpha"]) / 16.0)
    return (wq.astype(np.float32), wk.astype(np.float32), wv.astype(np.float32),
            wo.astype(np.float32))


def kernel(trace=False, **inputs):
    nc = _get_nc()
    x = np.ascontiguousarray(np.asarray(inputs["x"], np.float32))
    context = np.ascontiguousarray(np.asarray(inputs["context"], np.float32))
    wq, wk, wv, wo = _fold_weights(inputs)
    wki = np.asarray(inputs["Wk_ipa"], np.float32)
    wvi = np.asarray(inputs["Wv_ipa"], np.float32)
    bout = np.asarray(inputs["bout"], np.float32)

    # wqT head-major [DH, H, QD]
    wqt = np.ascontiguousarray(wq.T.reshape(H, DH, QD).transpose(1, 0, 2))
    # wo head-major [DH, H, QD]
    wo_hm = np.ascontiguousarray(wo.reshape(H, DH, QD).transpose(1, 0, 2))
    def chmajor(w):
        # [1536, 640] -> [P, 2, KBA, 320] (partition-major, col-chunk-major)
        return np.ascontiguousarray(
            w.reshape(KBA, P, 2, 320).transpose(1, 2, 0, 3))
    wkc = chmajor(np.concatenate([wk, wki], axis=0))
    wvc = chmajor(np.concatenate([wv, wvi], axis=0))
    boutb = bout.astype(ml_dtypes.bfloat16)

    sel16_h = np.zeros((CTXP, H, 16), np.float32)
    sel2t_h = np.zeros((17, H, CTXP), np.float32)
    for h in range(H):
        sel16_h[0:ORG, h, 2 * h] = 1.0
        sel16_h[96:CTXP, h, 2 * h + 1] = 1.0
        sel2t_h[2 * h, h, 0:ORG] = 1.0
        sel2t_h[2 * h + 1, h, 96:CTXP] = 1.0
    sel2t_h[16, 0, BIASROW] = 1.0
    sel16_b = sel16_h.astype(ml_dtypes.bfloat16)

    shared = dict(wqt=wqt, wkc=wkc, wvc=wvc, wo=wo_hm, boutb=boutb,
                  sel16=sel16_b, sel2t=sel2t_h,
                  identd=np.eye(P, dtype=np.float32),
                  zaug=np.zeros((P, KBA * CTXP), np.float32))
    in_maps = []
    for i in range(8):
        b, half = i // 2, i % 2
        in_maps.append(dict(
            x=np.ascontiguousarray(x[b, half * NSH:(half + 1) * NSH, :]),
            ctx=np.ascontiguousarray(context[b]),
            **shared,
        ))
    res = run_bass_kernel_spmd(nc, in_maps, list(range(8)), trace=trace)
    outp = np.empty((B, N, QD), np.float32)
    for i in range(8):
        b, half = i // 2, i % 2
        outp[b, half * NSH:(half + 1) * NSH, :] = np.asarray(
            res.results[i]["out"]).astype(np.float32)
    if trace:
        return outp, res
    return outp
